# revision 43
# baseline (speedup 1.0000x reference)
"""Trainium2 Bass kernel for AttLayer pooling (B=32, T=2048, D=1024, H=5). v12

Math identical to baseline kernel.py, with the mask fold restated:
    out[b,:] = sum_t x[b,t,:] * g[b,t] / sum_t g[b,t]
    g[b,t]   = exp(s[b,t] + 100*mask[b,t] - 100)   (masked: exp(s-100) -> 0)
    s[b,t]   = sum_h tanh( (x @ W)[b,t,h] + fea[b,t]*Wf[h] + bw[h] ) * uw[h]

v12 on top of v11:
  - x stream starts at t=0: batch-0 chunk-0 issued as 4x128-t sub-DMAs
    before any const/mask prep, so the first transposes start ~3us in.
  - dma_grp=512 (one compute group per chunk) with 6 x-pool bufs: finer
    DMA/PE pipelining, no special big-chunk case.
  - fea path in bf16 via cast DMA (f32r rank-1 matmuls were 2.2x slower).
  - mask rows: per-batch u8->f32 cast DMA, then one ACT copy straight
    into tanh_b row 5 (drops the bf16 staging row + SBUF->SBUF DMA).
"""

import sys

sys.path.insert(0, "/opt/trn_rl_repo")

import numpy as np

import concourse.bass as bass
import concourse.mybir as mybir
import concourse.tile as tile
from concourse import bacc
from concourse.masks import make_identity

F32 = mybir.dt.float32
F32R = mybir.dt.float32r
BF16 = mybir.dt.bfloat16
U8 = mybir.dt.uint8
AF = mybir.ActivationFunctionType

P = 128          # partitions / t-tile size
D = 1024         # feature dim
H = 5            # attention hidden dim
NCHUNK = D // P  # 8 d-chunks per tile


def build_kernel(b_shard: int, T: int, t_grp: int = 512):
    assert t_grp % P == 0 and T % t_grp == 0
    dma_grp = t_grp           # one compute group per DMA chunk
    jg = t_grp // P           # tiles per compute group
    jd = dma_grp // P         # tiles per DMA chunk
    n_dma = T // dma_grp
    n_tiles = T // P

    nc = bacc.Bacc(None)

    x_temp = nc.dram_tensor("x_temp", [b_shard, T, D], F32, kind="ExternalInput")
    x_fea = nc.dram_tensor("x_fea", [b_shard, T], F32, kind="ExternalInput")
    mask = nc.dram_tensor("mask", [b_shard, T], U8, kind="ExternalInput")
    W_temp = nc.dram_tensor("W_temp", [D, H], F32, kind="ExternalInput")
    W_fea = nc.dram_tensor("W_fea", [1, H], F32, kind="ExternalInput")
    bw = nc.dram_tensor("bw", [H], F32, kind="ExternalInput")
    uw = nc.dram_tensor("uw", [H], F32, kind="ExternalInput")
    out = nc.dram_tensor("out", [b_shard, D], F32, kind="ExternalOutput")

    with tile.TileContext(nc) as tc:
        with (
            tc.tile_pool(name="consts", bufs=1) as consts,
            tc.tile_pool(name="xpool", bufs=6) as xpool,
            tc.tile_pool(name="xtpool", bufs=6) as xtpool,
            tc.tile_pool(name="rows", bufs=2) as rows,
            tc.tile_pool(name="small", bufs=2) as small,
            tc.tile_pool(name="tp_ps", bufs=3, space="PSUM") as tp_ps,
            tc.tile_pool(name="sc_ps", bufs=1, space="PSUM") as sc_ps,
            tc.tile_pool(name="g_ps", bufs=1, space="PSUM") as g_ps,
            tc.tile_pool(name="acc_ps", bufs=1, space="PSUM") as acc_ps,
        ):
            # ---- identity first (first transpose gates only on data) ----
            ident = consts.tile([P, P], BF16)
            make_identity(nc, ident[:])

            # ---- x stream head: first chunk issued before other consts,
            # split by d-quarters to match transpose consumption order ----
            x3_first = xpool.tile([P, jd, D], BF16, tag="x")
            for q in range(4):
                nc.gpsimd.dma_start(
                    x3_first[:, :, q * 256 : (q + 1) * 256],
                    x_temp[0, 0:t_grp, q * 256 : (q + 1) * 256].rearrange(
                        "(j p) d -> p j d", p=P
                    ),
                )
            w_f = consts.tile([P, NCHUNK, H], F32)
            nc.sync.dma_start(w_f[:], W_temp.rearrange("(c p) h -> p c h", p=P))
            w_sb = consts.tile([P, NCHUNK, H], BF16)
            nc.vector.tensor_copy(w_sb[:], w_f[:])
            wfc = consts.tile([H, 1], F32)
            nc.sync.dma_start(wfc[:], W_fea[0, :, None])
            bw_sb = consts.tile([H, 1], F32)
            nc.sync.dma_start(bw_sb[:], bw[:, None])
            # uw_aug = [uw; 100.0] (mask fold: row 5 = raw mask 0/1; exp gets
            # bias=-100 so masked lanes underflow to zero)
            uwa_f = consts.tile([H + 1, 2], F32)
            nc.vector.memset(uwa_f[:], 100.0)
            nc.sync.dma_start(uwa_f[:H, 0:1], uw[:, None])
            nc.sync.dma_start(uwa_f[:H, 1:2], uw[:, None])
            uwa_sb = consts.tile([H + 1, 2], BF16)
            nc.vector.tensor_copy(uwa_sb[:], uwa_f[:])
            ones_sb = consts.tile([P, 1], BF16)
            nc.vector.memset(ones_sb[:], 1.0)
            nbias_sb = consts.tile([P, 1], F32)
            nc.vector.memset(nbias_sb[:], -100.0)

            # pending num/den work for the previous group:
            # (xb, g_sb, grp_idx, nm, den)
            pending = None
            tail = None        # deferred uw+exp emission for the previous group
            finalize = None    # deferred normalize/store for the previous batch

            def emit_pending_item(pend, i):
                xb_, gsb_, ti0_, nm_, den_, _ = pend
                j = i // 3
                k = i % 3
                tt = ti0_ + j
                first = tt == 0
                last = tt == n_tiles - 1
                if k < 2:
                    nc.tensor.matmul(
                        nm_[32:33, k * 512 : (k + 1) * 512],
                        gsb_[:, tt : tt + 1],
                        xb_[:, j, k * 512 : (k + 1) * 512],
                        start=first,
                        stop=last,
                        tile_position=(0, 32),
                    )
                else:
                    nc.tensor.matmul(
                        den_[32:33, :],
                        gsb_[:, tt : tt + 1],
                        ones_sb[:],
                        start=first,
                        stop=last,
                        tile_position=(0, 32),
                    )

            for b in range(b_shard):
                # ---- per-batch rows ----
                # fea replicated to H partitions (plain HWDGE DMAs, no cast);
                # its Wf product folds into one DVE op per group
                fea_rep = rows.tile([H, T], F32, tag="fea")
                for h in range(H):
                    nc.sync.dma_start(fea_rep[h : h + 1, :], x_fea[b : b + 1, :])
                tanh_b = rows.tile([H + 1, T], BF16, tag="tanhb")
                # row 5 = raw mask (0/1), u8 -> bf16 cast DMA, exact
                nc.gpsimd.dma_start(tanh_b[H : H + 1, :], mask[b : b + 1, :])
                g_sb = rows.tile([P, n_tiles], BF16, tag="gsb")

                nm = acc_ps.tile([33, D], F32, tag="num")
                den = acc_ps.tile([33, 1], F32, tag="den")

                # last batch tapers off with two 256-t groups so the final
                # serial chain (scores->tanh->exp->num) is half as deep
                if b == b_shard - 1:
                    segs = [(gi * t_grp, t_grp) for gi in range(n_dma - 1)]
                    segs += [(T - t_grp, t_grp // 2), (T - t_grp // 2, t_grp // 2)]
                else:
                    segs = [(gi * t_grp, t_grp) for gi in range(n_dma)]

                for t0, tg in segs:
                    jgs = tg // P
                    ti0 = t0 // P
                    if b == 0 and t0 == 0:
                        xb = x3_first
                    else:
                        xb = xpool.tile([P, tg // P, D], BF16, tag="x")
                        nc.gpsimd.dma_start(
                            xb[:],
                            x_temp[b, t0 : t0 + tg, :].rearrange(
                                "(j p) d -> p j d", p=P
                            ),
                        )
                    # ---- transposes + DVE copies; prev group's uw+exp
                    # emitted between the two transpose halves ----
                    xts = []
                    for cp in range(NCHUNK // 2):
                        if cp == 2 and tail is not None:
                            tail()
                            tail = None
                        tp = tp_ps.tile([P, 2, tg], BF16, tag="tp")
                        for c2 in range(2):
                            c = cp * 2 + c2
                            for j in range(jgs):
                                nc.tensor.transpose(
                                    tp[:, c2, j * P : (j + 1) * P],
                                    xb[:, j, c * P : (c + 1) * P],
                                    ident[:],
                                )
                        xt = xtpool.tile([P, 2, tg], BF16, tag="xt")
                        nc.vector.tensor_copy(xt[:], tp[:])
                        xts.append(xt)
                    if tail is not None:
                        tail()
                        tail = None
                    # ---- scores stream (strip 0) with pending num/den
                    # (strip 1) interleaved => concurrent on the PE; fea
                    # rank-1 last so it runs warm ----
                    sc = sc_ps.tile([H, tg], F32, tag="sc")
                    pi = 0
                    n_pend_cur = pending[5] * 3 if pending is not None else 0
                    for c in range(NCHUNK):
                        if pending is not None:
                            take = 2 if c < (n_pend_cur - NCHUNK) else 1
                            for _ in range(take):
                                if pi < n_pend_cur:
                                    emit_pending_item(pending, pi)
                                    pi += 1
                        nc.tensor.matmul(
                            sc[:],
                            w_sb[:, c, :],
                            xts[c // 2][:, c % 2, :],
                            start=(c == 0),
                            stop=(c == NCHUNK - 1),
                        )
                    if pending is not None:
                        while pi < n_pend_cur:
                            emit_pending_item(pending, pi)
                            pi += 1
                        pending = None
                    # previous batch fully accumulated -> normalize/store
                    if finalize is not None:
                        finalize()
                        finalize = None
                    # sc += fea * Wf (one fused DVE op, off the PE)
                    nc.vector.scalar_tensor_tensor(
                        sc[:],
                        fea_rep[:, t0 : t0 + tg],
                        wfc[:],
                        sc[:],
                        mybir.AluOpType.mult,
                        mybir.AluOpType.add,
                    )
                    # tanh(sc + bw) -> tanh_b rows 0..4 (ACT, off PE path)
                    nc.scalar.activation(
                        tanh_b[:H, t0 : t0 + tg], sc[:], AF.Tanh, bias=bw_sb[:]
                    )

                    def make_tail(t0=t0, ti0=ti0, jgs=jgs, g_sb=g_sb, tanh_b=tanh_b):
                        def tail_fn():
                            gp = g_ps.tile([P, jgs, 2], F32, tag="g")
                            for j in range(jgs):
                                nc.tensor.matmul(
                                    gp[:, j, :],
                                    tanh_b[:, t0 + j * P : t0 + (j + 1) * P],
                                    uwa_sb[:],
                                    start=True,
                                    stop=True,
                                )
                            nc.scalar.activation(
                                g_sb[:, ti0 : ti0 + jgs],
                                gp[:, :, 0],
                                AF.Exp,
                                bias=nbias_sb[:],
                            )
                        return tail_fn

                    tail = make_tail()
                    pending = (xb, g_sb, ti0, nm, den, jgs)

                # defer normalize/store: pending of the last group drains in
                # the next batch's first scores stream
                def make_finalize(b=b, nm=nm, den=den):
                    def fin():
                        inv = small.tile([33, 1], F32, tag="inv")
                        nc.vector.reciprocal(inv[32:33, :], den[32:33, :])
                        o_sb = small.tile([33, D], F32, tag="osb")
                        # halves on DVE + ACT in parallel
                        nc.vector.tensor_scalar_mul(
                            o_sb[32:33, : D // 2],
                            nm[32:33, : D // 2],
                            inv[32:33, :],
                        )
                        nc.scalar.activation(
                            o_sb[32:33, D // 2 :],
                            nm[32:33, D // 2 :],
                            AF.Copy,
                            scale=inv[32:33, :],
                        )
                        nc.sync.dma_start(out[b : b + 1, :], o_sb[32:33, :])
                    return fin

                finalize = make_finalize()

            # drain the last batch
            if tail is not None:
                tail()
                tail = None
            if pending is not None:
                for i in range(pending[5] * 3):
                    emit_pending_item(pending, i)
                pending = None
            if finalize is not None:
                finalize()
                finalize = None

    nc.finalize()
    return nc


_NC_CACHE = {}


def _get_nc(b_shard, T):
    key = (b_shard, T)
    if key not in _NC_CACHE:
        _NC_CACHE[key] = build_kernel(b_shard, T)
    return _NC_CACHE[key]


def kernel(x_temp, x_fea, mask, W_temp, W_fea, bw, uw) -> np.ndarray:
    from concourse.bass_utils import run_bass_kernel_spmd

    B, T, D_ = x_temp.shape
    n_cores = 8
    assert B % n_cores == 0
    bs = B // n_cores

    nc = _get_nc(bs, T)

    x_temp = np.ascontiguousarray(x_temp, dtype=np.float32)
    x_fea = np.ascontiguousarray(x_fea, dtype=np.float32)
    mask_u8 = np.ascontiguousarray(mask).view(np.uint8)
    W_temp = np.ascontiguousarray(W_temp, dtype=np.float32)
    W_fea = np.ascontiguousarray(W_fea, dtype=np.float32)
    bw = np.ascontiguousarray(bw, dtype=np.float32)
    uw = np.ascontiguousarray(uw, dtype=np.float32)

    in_maps = []
    for i in range(n_cores):
        in_maps.append(
            {
                "x_temp": x_temp[i * bs : (i + 1) * bs],
                "x_fea": x_fea[i * bs : (i + 1) * bs],
                "mask": mask_u8[i * bs : (i + 1) * bs],
                "W_temp": W_temp,
                "W_fea": W_fea,
                "bw": bw,
                "uw": uw,
            }
        )

    res = run_bass_kernel_spmd(nc, in_maps, core_ids=list(range(n_cores)))
    return np.concatenate([r["out"] for r in res.results], axis=0)


# revision 44
# speedup vs baseline: 1.1329x; 1.1329x over previous
"""Trainium2 Bass kernel for AttLayer pooling (B=32, T=2048, D=1024, H=5). v16

Math identical to the reference, with the mask fold restated:
    out[b,:] = sum_t x[b,t,:] * g[b,t] / sum_t g[b,t]
    g[b,t]   = exp(s[b,t] + 100*mask[b,t] - 100)   (masked: exp(s-100) -> 0)
    s[b,t]   = sum_h tanh( (x @ W)[b,t,h] + fea[b,t]*Wf[h] + bw[h] ) * uw[h]

Per core (b_shard=4): stream x in 512-t chunks (DMA f32->bf16 cast),
PE-transpose each 128x128 tile to PSUM, DVE-copy to SBUF, accumulate
scores sc[5,512] = W^T x^T on the PE with the PREVIOUS group's pooling
matmuls (num = g^T x, den = g^T 1) interleaved between the accumulating
scores matmuls.  tanh/exp run on ACT off the PE path; the mask fold
costs zero extra ops (row 5 of the tanh tile = raw mask via a u8->bf16
cast DMA, uw_aug row = 100.0, exp bias = -100).

Highlights over the staged v11 baseline (139us -> ~115us typical):
  - x stream starts at t=0: chunk 0 issued before all consts, split by
    d-quarters to match transpose consumption order.
  - one compute group per DMA chunk (512 t) with 6 x-pool bufs.
  - fea*Wf off the PE entirely: fea replicated to 5 partitions by plain
    HWDGE DMAs, folded into sc with one fused DVE scalar_tensor_tensor.
  - last batch tapers into two 256-t groups; finalize split DVE/ACT.

Rejected by interleaved on-HW A/B (each lost 4+ of 5 pairs): one-group
software-pipeline skew of transposes vs scores (longer in-order PE
tail); dual-PSUM-bank scores accumulation w/ tp_ps=2 (transpose stalls
outweigh the accumulate-turnaround savings); per-chunk (finer) DVE
copies; ACT-assisted PSUM->SBUF copies; replacing den matmuls with ACT
accum_out (den matmuls are useful PE filler); PE clock warmup bursts.
"""

import sys

sys.path.insert(0, "/opt/trn_rl_repo")

import numpy as np

import concourse.bass as bass
import concourse.mybir as mybir
import concourse.tile as tile
from concourse import bacc
from concourse.masks import make_identity

F32 = mybir.dt.float32
F32R = mybir.dt.float32r
BF16 = mybir.dt.bfloat16
U8 = mybir.dt.uint8
AF = mybir.ActivationFunctionType

P = 128          # partitions / t-tile size
D = 1024         # feature dim
H = 5            # attention hidden dim
NCHUNK = D // P  # 8 d-chunks per tile


def build_kernel(b_shard: int, T: int, t_grp: int = 512):
    assert t_grp % P == 0 and T % t_grp == 0
    dma_grp = t_grp           # one compute group per DMA chunk
    jg = t_grp // P           # tiles per compute group
    jd = dma_grp // P         # tiles per DMA chunk
    n_dma = T // dma_grp
    n_tiles = T // P

    nc = bacc.Bacc(None)

    x_temp = nc.dram_tensor("x_temp", [b_shard, T, D], F32, kind="ExternalInput")
    x_fea = nc.dram_tensor("x_fea", [b_shard, T], F32, kind="ExternalInput")
    mask = nc.dram_tensor("mask", [b_shard, T], U8, kind="ExternalInput")
    W_temp = nc.dram_tensor("W_temp", [D, H], F32, kind="ExternalInput")
    W_fea = nc.dram_tensor("W_fea", [1, H], F32, kind="ExternalInput")
    bw = nc.dram_tensor("bw", [H], F32, kind="ExternalInput")
    uw = nc.dram_tensor("uw", [H], F32, kind="ExternalInput")
    out = nc.dram_tensor("out", [b_shard, D], F32, kind="ExternalOutput")

    with tile.TileContext(nc) as tc:
        with (
            tc.tile_pool(name="consts", bufs=1) as consts,
            tc.tile_pool(name="xpool", bufs=6) as xpool,
            tc.tile_pool(name="xtpool", bufs=6) as xtpool,
            tc.tile_pool(name="rows", bufs=2) as rows,
            tc.tile_pool(name="small", bufs=2) as small,
            tc.tile_pool(name="tp_ps", bufs=3, space="PSUM") as tp_ps,
            tc.tile_pool(name="sc_ps", bufs=1, space="PSUM") as sc_ps,
            tc.tile_pool(name="g_ps", bufs=1, space="PSUM") as g_ps,
            tc.tile_pool(name="acc_ps", bufs=1, space="PSUM") as acc_ps,
        ):
            # ---- identity first (first transpose gates only on data) ----
            ident = consts.tile([P, P], BF16)
            make_identity(nc, ident[:])

            # ---- x stream head: first chunk issued before other consts,
            # split by d-quarters to match transpose consumption order ----
            x3_first = xpool.tile([P, jd, D], BF16, tag="x")
            for q in range(4):
                nc.gpsimd.dma_start(
                    x3_first[:, :, q * 256 : (q + 1) * 256],
                    x_temp[0, 0:t_grp, q * 256 : (q + 1) * 256].rearrange(
                        "(j p) d -> p j d", p=P
                    ),
                )
            w_f = consts.tile([P, NCHUNK, H], F32)
            nc.sync.dma_start(w_f[:], W_temp.rearrange("(c p) h -> p c h", p=P))
            w_sb = consts.tile([P, NCHUNK, H], BF16)
            nc.vector.tensor_copy(w_sb[:], w_f[:])
            wfc = consts.tile([H, 1], F32)
            nc.sync.dma_start(wfc[:], W_fea[0, :, None])
            bw_sb = consts.tile([H, 1], F32)
            nc.sync.dma_start(bw_sb[:], bw[:, None])
            # uw_aug = [uw; 100.0] (mask fold: row 5 = raw mask 0/1; exp gets
            # bias=-100 so masked lanes underflow to zero)
            uwa_f = consts.tile([H + 1, 2], F32)
            nc.vector.memset(uwa_f[:], 100.0)
            nc.sync.dma_start(uwa_f[:H, 0:1], uw[:, None])
            nc.sync.dma_start(uwa_f[:H, 1:2], uw[:, None])
            uwa_sb = consts.tile([H + 1, 2], BF16)
            nc.vector.tensor_copy(uwa_sb[:], uwa_f[:])
            ones_sb = consts.tile([P, 1], BF16)
            nc.vector.memset(ones_sb[:], 1.0)
            nbias_sb = consts.tile([P, 1], F32)
            nc.vector.memset(nbias_sb[:], -100.0)

            # pending num/den work for the previous group:
            # (xb, g_sb, grp_idx, nm, den)
            pending = None
            tail = None        # deferred uw+exp emission for the previous group
            finalize = None    # deferred normalize/store for the previous batch

            def emit_pending_item(pend, i):
                xb_, gsb_, ti0_, nm_, den_, _ = pend
                j = i // 3
                k = i % 3
                tt = ti0_ + j
                first = tt == 0
                last = tt == n_tiles - 1
                if k < 2:
                    nc.tensor.matmul(
                        nm_[32:33, k * 512 : (k + 1) * 512],
                        gsb_[:, tt : tt + 1],
                        xb_[:, j, k * 512 : (k + 1) * 512],
                        start=first,
                        stop=last,
                        tile_position=(0, 32),
                    )
                else:
                    nc.tensor.matmul(
                        den_[32:33, :],
                        gsb_[:, tt : tt + 1],
                        ones_sb[:],
                        start=first,
                        stop=last,
                        tile_position=(0, 32),
                    )

            for b in range(b_shard):
                # ---- per-batch rows ----
                # fea replicated to H partitions (plain HWDGE DMAs, no cast);
                # its Wf product folds into one DVE op per group
                fea_rep = rows.tile([H, T], F32, tag="fea")
                for h in range(H):
                    nc.sync.dma_start(fea_rep[h : h + 1, :], x_fea[b : b + 1, :])
                tanh_b = rows.tile([H + 1, T], BF16, tag="tanhb")
                # row 5 = raw mask (0/1), u8 -> bf16 cast DMA, exact
                nc.gpsimd.dma_start(tanh_b[H : H + 1, :], mask[b : b + 1, :])
                g_sb = rows.tile([P, n_tiles], BF16, tag="gsb")

                nm = acc_ps.tile([33, D], F32, tag="num")
                den = acc_ps.tile([33, 1], F32, tag="den")

                # last batch tapers off with two 256-t groups so the final
                # serial chain (scores->tanh->exp->num) is half as deep
                if b == b_shard - 1:
                    segs = [(gi * t_grp, t_grp) for gi in range(n_dma - 1)]
                    segs += [(T - t_grp, t_grp // 2), (T - t_grp // 2, t_grp // 2)]
                else:
                    segs = [(gi * t_grp, t_grp) for gi in range(n_dma)]

                for t0, tg in segs:
                    jgs = tg // P
                    ti0 = t0 // P
                    if b == 0 and t0 == 0:
                        xb = x3_first
                    else:
                        xb = xpool.tile([P, tg // P, D], BF16, tag="x")
                        nc.gpsimd.dma_start(
                            xb[:],
                            x_temp[b, t0 : t0 + tg, :].rearrange(
                                "(j p) d -> p j d", p=P
                            ),
                        )
                    # ---- transposes + DVE copies; prev group's uw+exp
                    # emitted between the two transpose halves ----
                    xts = []
                    for cp in range(NCHUNK // 2):
                        if cp == 2 and tail is not None:
                            tail()
                            tail = None
                        tp = tp_ps.tile([P, 2, tg], BF16, tag="tp")
                        for c2 in range(2):
                            c = cp * 2 + c2
                            for j in range(jgs):
                                nc.tensor.transpose(
                                    tp[:, c2, j * P : (j + 1) * P],
                                    xb[:, j, c * P : (c + 1) * P],
                                    ident[:],
                                )
                        xt = xtpool.tile([P, 2, tg], BF16, tag="xt")
                        nc.vector.tensor_copy(xt[:], tp[:])
                        xts.append(xt)
                    if tail is not None:
                        tail()
                        tail = None
                    # ---- scores stream (strip 0) with pending num/den
                    # (strip 1) interleaved => concurrent on the PE; fea
                    # rank-1 last so it runs warm ----
                    sc = sc_ps.tile([H, tg], F32, tag="sc")
                    pi = 0
                    n_pend_cur = pending[5] * 3 if pending is not None else 0
                    for c in range(NCHUNK):
                        if pending is not None:
                            take = 2 if c < (n_pend_cur - NCHUNK) else 1
                            for _ in range(take):
                                if pi < n_pend_cur:
                                    emit_pending_item(pending, pi)
                                    pi += 1
                        nc.tensor.matmul(
                            sc[:],
                            w_sb[:, c, :],
                            xts[c // 2][:, c % 2, :],
                            start=(c == 0),
                            stop=(c == NCHUNK - 1),
                        )
                    if pending is not None:
                        while pi < n_pend_cur:
                            emit_pending_item(pending, pi)
                            pi += 1
                        pending = None
                    # previous batch fully accumulated -> normalize/store
                    if finalize is not None:
                        finalize()
                        finalize = None
                    # sc += fea * Wf (one fused DVE op, off the PE)
                    nc.vector.scalar_tensor_tensor(
                        sc[:],
                        fea_rep[:, t0 : t0 + tg],
                        wfc[:],
                        sc[:],
                        mybir.AluOpType.mult,
                        mybir.AluOpType.add,
                    )
                    # tanh(sc + bw) -> tanh_b rows 0..4 (ACT, off PE path)
                    nc.scalar.activation(
                        tanh_b[:H, t0 : t0 + tg], sc[:], AF.Tanh, bias=bw_sb[:]
                    )

                    def make_tail(t0=t0, ti0=ti0, jgs=jgs, g_sb=g_sb, tanh_b=tanh_b):
                        def tail_fn():
                            gp = g_ps.tile([P, jgs, 2], F32, tag="g")
                            for j in range(jgs):
                                nc.tensor.matmul(
                                    gp[:, j, :],
                                    tanh_b[:, t0 + j * P : t0 + (j + 1) * P],
                                    uwa_sb[:],
                                    start=True,
                                    stop=True,
                                )
                            nc.scalar.activation(
                                g_sb[:, ti0 : ti0 + jgs],
                                gp[:, :, 0],
                                AF.Exp,
                                bias=nbias_sb[:],
                            )
                        return tail_fn

                    tail = make_tail()
                    pending = (xb, g_sb, ti0, nm, den, jgs)

                # defer normalize/store: pending of the last group drains in
                # the next batch's first scores stream
                def make_finalize(b=b, nm=nm, den=den):
                    def fin():
                        inv = small.tile([33, 1], F32, tag="inv")
                        nc.vector.reciprocal(inv[32:33, :], den[32:33, :])
                        o_sb = small.tile([33, D], F32, tag="osb")
                        # halves on DVE + ACT in parallel
                        nc.vector.tensor_scalar_mul(
                            o_sb[32:33, : D // 2],
                            nm[32:33, : D // 2],
                            inv[32:33, :],
                        )
                        nc.scalar.activation(
                            o_sb[32:33, D // 2 :],
                            nm[32:33, D // 2 :],
                            AF.Copy,
                            scale=inv[32:33, :],
                        )
                        nc.sync.dma_start(out[b : b + 1, :], o_sb[32:33, :])
                    return fin

                finalize = make_finalize()

            # drain the last batch
            if tail is not None:
                tail()
                tail = None
            if pending is not None:
                for i in range(pending[5] * 3):
                    emit_pending_item(pending, i)
                pending = None
            if finalize is not None:
                finalize()
                finalize = None

    nc.finalize()
    return nc


_NC_CACHE = {}


def _get_nc(b_shard, T):
    key = (b_shard, T)
    if key not in _NC_CACHE:
        _NC_CACHE[key] = build_kernel(b_shard, T)
    return _NC_CACHE[key]


def kernel(x_temp, x_fea, mask, W_temp, W_fea, bw, uw) -> np.ndarray:
    from concourse.bass_utils import run_bass_kernel_spmd

    B, T, D_ = x_temp.shape
    n_cores = 8
    assert B % n_cores == 0
    bs = B // n_cores

    nc = _get_nc(bs, T)

    x_temp = np.ascontiguousarray(x_temp, dtype=np.float32)
    x_fea = np.ascontiguousarray(x_fea, dtype=np.float32)
    mask_u8 = np.ascontiguousarray(mask).view(np.uint8)
    W_temp = np.ascontiguousarray(W_temp, dtype=np.float32)
    W_fea = np.ascontiguousarray(W_fea, dtype=np.float32)
    bw = np.ascontiguousarray(bw, dtype=np.float32)
    uw = np.ascontiguousarray(uw, dtype=np.float32)

    in_maps = []
    for i in range(n_cores):
        in_maps.append(
            {
                "x_temp": x_temp[i * bs : (i + 1) * bs],
                "x_fea": x_fea[i * bs : (i + 1) * bs],
                "mask": mask_u8[i * bs : (i + 1) * bs],
                "W_temp": W_temp,
                "W_fea": W_fea,
                "bw": bw,
                "uw": uw,
            }
        )

    res = run_bass_kernel_spmd(nc, in_maps, core_ids=list(range(n_cores)))
    return np.concatenate([r["out"] for r in res.results], axis=0)
